# revision 8
# baseline (speedup 1.0000x reference)
"""Trainium2 Bass kernel for nn_HSIM_27771258536586 (histogram_binning).

score = sum_{b,k} min(p,t)/(p + (p==0)) / (B*BINS) over KDE histograms
p,t of pred/target, 30 gaussian bins on [0,1].

Key algorithmic facts exploited:
 - exp(-0.5*((x-c)/delta)^2) == sqrt(pi)/2 * Derivative_Erf((30x - z_b)/sqrt(2))
   and the final score is invariant to any positive rescale of BOTH
   histograms, so the 2/sqrt(pi) constant never needs correcting.
 - ACT's `accum_out` gives the per-partition running sum of the activation
   output in the same single pass, so one bin costs exactly one ACT
   instruction over the core's data; no separate reduce pass needed.

Sharding: data-parallel over B: core c computes the full histogram pair for
batch c (pred[c] on SBUF partitions 0..63, target[c] on partitions 64..127),
its partial score sum_b min/p / 240, then an AllGather + on-device sum
produces the full scalar on every core.
"""

import math

import numpy as np

import concourse.bass as bass
import concourse.mybir as mybir
import concourse.tile as tile
from concourse import bacc, bass_utils

N_CORES = 8
BINS = 30
PP = 64            # pred partitions (target: 64..127)
FC = 2352          # 3*224*224 / 64
F32 = mybir.dt.float32
SQ2 = math.sqrt(2.0)

_cache = {}


def _build(use_collective: bool = True):
    nc = bacc.Bacc(
        "TRN2", target_bir_lowering=False, debug=False, num_devices=N_CORES
    )
    pred_d = nc.dram_tensor("pred", [PP, FC], F32, kind="ExternalInput")
    targ_d = nc.dram_tensor("target", [PP, FC], F32, kind="ExternalInput")
    out_d = nc.dram_tensor("out", [1, 1], F32, kind="ExternalOutput")

    with tile.TileContext(nc) as tc:
        with (
            tc.tile_pool(name="data", bufs=1) as data_pool,
            tc.tile_pool(name="scratch", bufs=2) as scratch_pool,
            tc.tile_pool(name="small", bufs=1) as small_pool,
            tc.tile_pool(name="psum", bufs=1, space="PSUM") as psum_pool,
            tc.tile_pool(name="dram", bufs=1, space="DRAM") as dram_pool,
        ):
            x = data_pool.tile([128, FC], F32)
            nc.sync.dma_start(x[0:PP, :], pred_d[:])
            nc.sync.dma_start(x[PP:128, :], targ_d[:])

            # tiny activation on a const tile: forces the ACT table load to
            # happen during the input DMA instead of after it
            warm = small_pool.tile([1, 2], F32)
            nc.vector.memset(warm[:], 0.0)
            warm2 = small_pool.tile([1, 2], F32)
            nc.scalar.activation(
                warm2[:], warm[:],
                mybir.ActivationFunctionType.Derivative_Erf,
                bias=0.0, scale=1.0,
            )

            # selector weights: col0 = pred rows, col1 = target rows
            sel = small_pool.tile([128, 2], F32)
            nc.vector.memset(sel[:], 0.0)
            nc.vector.memset(sel[0:PP, 0:1], 1.0)
            nc.vector.memset(sel[PP:128, 1:2], 1.0)
            ones8 = small_pool.tile([128, 1], F32)
            nc.vector.memset(ones8[:], 1.0)

            # per-bin bias values as an SBUF tile (bias APs must be [P,1]).
            # Built by ONE writer chain (iota -> cast -> fused affine) so the
            # 30 ACT passes need a single cross-engine wait instead of one
            # per-pass EventSemaphore (~187ns each on the ACT sequencer).
            bias_i = small_pool.tile([128, BINS], mybir.dt.int32)
            nc.gpsimd.iota(bias_i[:], pattern=[[1, BINS]], base=0,
                           channel_multiplier=0)
            bias_f = small_pool.tile([128, BINS], F32)
            nc.vector.tensor_copy(bias_f[:], bias_i[:])
            bias_t = small_pool.tile([128, BINS], F32)
            nc.vector.tensor_scalar(
                bias_t[:], bias_f[:], float(-1.0 / SQ2), float(-0.5 / SQ2),
                op0=mybir.AluOpType.mult, op1=mybir.AluOpType.add,
            )

            # 30 bins: one ACT pass each; accum_out -> column b of R.
            R = small_pool.tile([128, BINS], F32)
            for b in range(BINS):
                dummy = scratch_pool.tile([128, FC], F32, tag="dummy")
                nc.scalar.activation(
                    dummy[:],
                    x[:],
                    mybir.ActivationFunctionType.Derivative_Erf,
                    bias=bias_t[:, b : b + 1],
                    scale=float(30.0 / SQ2),
                    accum_out=R[:, b : b + 1],
                )

            # partition-sum R separately for pred/target rows via selector MMs
            pt = psum_pool.tile([1, 64], F32)
            nc.tensor.matmul(
                pt[0:1, 0:BINS], sel[:, 0:1], R[:, 0:BINS], start=True, stop=True
            )
            nc.tensor.matmul(
                pt[0:1, 32 : 32 + BINS],
                sel[:, 1:2],
                R[:, 0:BINS],
                start=True,
                stop=True,
            )

            ptc = small_pool.tile([1, 64], F32)
            nc.vector.tensor_copy(ptc[:], pt[:])
            P = ptc[0:1, 0:BINS]
            T = ptc[0:1, 32 : 32 + BINS]

            m = small_pool.tile([1, BINS], F32)
            nc.vector.tensor_tensor(m[:], P, T, op=mybir.AluOpType.min)
            mask = small_pool.tile([1, BINS], F32)
            nc.vector.tensor_scalar(
                mask[:], P, 0.0, None, op0=mybir.AluOpType.is_equal
            )
            pd = small_pool.tile([1, BINS], F32)
            nc.vector.tensor_tensor(pd[:], P, mask[:], op=mybir.AluOpType.add)
            rec = small_pool.tile([1, BINS], F32)
            nc.vector.reciprocal(rec[:], pd[:])
            q = small_pool.tile([1, BINS], F32)
            nc.vector.tensor_tensor(q[:], m[:], rec[:], op=mybir.AluOpType.mult)

            s = small_pool.tile([1, 1], F32)
            nc.vector.reduce_sum(s[:], q[:], axis=mybir.AxisListType.X)
            partial = small_pool.tile([1, 8], F32)
            nc.vector.memset(partial[:], 0.0)
            nc.vector.tensor_scalar(
                partial[0:1, 0:1], s[:], 1.0 / (8.0 * BINS), None,
                op0=mybir.AluOpType.mult,
            )

            if use_collective:
                cin = dram_pool.tile([1, 8], F32)
                cout = dram_pool.tile([8, 8], F32)
                nc.gpsimd.dma_start(cin[:], partial[:])
                nc.gpsimd.collective_compute(
                    "AllGather",
                    mybir.AluOpType.bypass,
                    replica_groups=[list(range(N_CORES))],
                    ins=[cin.opt()],
                    outs=[cout.opt()],
                )
                ag = small_pool.tile([8, 8], F32)
                nc.gpsimd.dma_start(ag[:], cout[:])
                fin = psum_pool.tile([1, 8], F32)
                nc.tensor.matmul(
                    fin[0:1, 0:1], ones8[0:8, 0:1], ag[0:8, 0:1],
                    start=True, stop=True,
                )
                fsb = small_pool.tile([1, 1], F32)
                nc.vector.tensor_copy(fsb[:], fin[0:1, 0:1])
                nc.gpsimd.dma_start(out_d[:], fsb[:])
            else:
                nc.gpsimd.dma_start(out_d[:], partial[0:1, 0:1])

    nc.compile()
    return nc


def _get(use_collective: bool = True):
    key = use_collective
    if key not in _cache:
        _cache[key] = _build(use_collective)
    return _cache[key]


def kernel(pred: np.ndarray, target: np.ndarray, _trace: bool = False):
    nc = _get(use_collective=True)
    pred = np.ascontiguousarray(pred, dtype=np.float32)
    target = np.ascontiguousarray(target, dtype=np.float32)
    in_maps = [
        {
            "pred": pred[c].reshape(PP, FC),
            "target": target[c].reshape(PP, FC),
        }
        for c in range(N_CORES)
    ]
    res = bass_utils.run_bass_kernel_spmd(
        nc, in_maps, core_ids=list(range(N_CORES)), trace=_trace
    )
    out = np.float32(res.results[0]["out"][0, 0])
    if _trace:
        kernel.last_result = res
    return np.asarray(out, dtype=np.float32)


if __name__ == "__main__":
    rng = np.random.default_rng(0)
    p = rng.random((8, 3, 224, 224), dtype=np.float32)
    t = rng.random((8, 3, 224, 224), dtype=np.float32)
    print("score:", kernel(p, t))
